# revision 1
# baseline (speedup 1.0000x reference)
"""Trainium2 Bass kernel for nn_AttentionBlock (GroupNorm + single-head
self-attention + proj + residual), data-parallel over batch on 8 cores.

Contract: kernel(**inputs) takes the FULL unsharded inputs
  x (8, 256, 64, 64) f32, gn_scale (256,), gn_bias (256,),
  qkv_w (768, 256), qkv_b (768,), proj_w (256, 256), proj_b (256,)
and returns the FULL output (8, 256, 64, 64) f32.

Per-core plan (one sample per core):
  - x viewed as (C=256, N=4096) = (channels on partitions, tokens on free dim)
  - GroupNorm(8 groups) stats via bn_stats/bn_aggr + tiny indicator matmuls
  - xn cast to bf16; QKV as channel matmuls:
      Q, K produced in (d, n) layout  [d on partitions]
      V produced in token-major (n, d) layout
    so that attention needs NO transposes:
      scoresT[k, q] = sum_d K[d,k] Q[d,q]   (lhsT=K tile, rhs=Q tile)
      PT = exp(scoresT)                     (softmax scale folded into Q)
      out_un[d, q] = sum_k V[k,d] PT[k,q]   (lhsT=V tile, rhs=PT)
      denom[q] = sum_k PT[k,q]  -> DVE accumulation + one ones-matmul that
                                   also broadcasts the sum over partitions
  - out = proj(out_un * 1/denom) + proj_b + x  (residual in f32)
"""

import os
import sys

import numpy as np

for _p in (
    "/opt/trn_rl_repo",
    "/root/.axon_site",
    "/root/.axon_site/_ro/trn_rl_repo",
    "/root/.axon_site/_ro/pypackages",
):
    if os.path.isdir(_p) and _p not in sys.path:
        sys.path.append(_p)

import ml_dtypes  # noqa: E402

import concourse.bass as bass  # noqa: E402
import concourse.mybir as mybir  # noqa: E402
import concourse.tile as tile  # noqa: E402
from concourse import bacc  # noqa: E402

F32 = mybir.dt.float32
BF16 = mybir.dt.bfloat16
FP8 = mybir.dt.float8e4
AF = mybir.ActivationFunctionType
ALU = mybir.AluOpType
DR = mybir.MatmulPerfMode.DoubleRow

B, C, H, W = 8, 256, 64, 64
GROUPS = 8
EPS = 1e-5
P = 128
N_CORES = 8
ATT_SCALE = float(C) ** -0.5  # 1/16


def build_nc(n_tok=H * W):
    """Build the single-core Bass program (SPMD across 8 cores)."""
    CCH = C // P            # channel chunks (2)
    QT = 512                # q-tile width (one PSUM bank of f32)
    NQ = n_tok // QT        # number of q tiles
    NKB = n_tok // P        # number of 128-token key blocks
    GSZ = C // GROUPS       # channels per group (32)
    G_PER_CHUNK = GROUPS // CCH  # groups per 128-channel chunk (4)

    # Bacc (not plain Bass): its compile() runs move_matmul_waits_to_ldweights
    # + generate_event_semaphores, which split multi-wait matmuls to satisfy
    # the 1-sync-wait-per-instruction hardware constraint.
    nc = bacc.Bacc()

    # ---- DRAM I/O (per-core tensors; host shards batch over cores) ----
    x_d = nc.dram_tensor("x", [C, n_tok], F32, kind="ExternalInput")
    qkvw_d = nc.dram_tensor("qkv_wt", [CCH, P, 3 * C], BF16, kind="ExternalInput")
    qkbias_d = nc.dram_tensor("qk_bias", [4, P, 1], F32, kind="ExternalInput")
    vbias_d = nc.dram_tensor("v_bias", [C], F32, kind="ExternalInput")
    projw_d = nc.dram_tensor("proj_wt", [CCH, P, C], BF16, kind="ExternalInput")
    projb_d = nc.dram_tensor("proj_b", [CCH, P, 1], F32, kind="ExternalInput")
    gnsc_d = nc.dram_tensor("gn_sc", [CCH, P, 1], F32, kind="ExternalInput")
    gnbi_d = nc.dram_tensor("gn_bi", [CCH, P, 1], F32, kind="ExternalInput")
    # group-sum indicator (zero-padded to M=128 so the matmul avoids the
    # 32-column tile-mode lowering): ind[t, c, g] = (t*128 + c) // 32 == g
    gnind_d = nc.dram_tensor("gn_ind", [CCH, P, P], F32, kind="ExternalInput")
    # channel-broadcast indicator, padded to K=128: ind2[t, g, c] nonzero only g<8
    gnind2_d = nc.dram_tensor("gn_ind2", [CCH, P, P], F32, kind="ExternalInput")
    out_d = nc.dram_tensor("out", [C, n_tok], F32, kind="ExternalOutput")

    QP = 2 * QT             # paired q-tile width (1024)
    NQP = n_tok // QP       # number of q-tile pairs

    with tile.TileContext(nc) as tc:
        with (
            tc.tile_pool(name="persist", bufs=1) as pp,
            tc.tile_pool(name="work", bufs=3) as wp,
            tc.tile_pool(name="ps_sc", bufs=2, space="PSUM") as psb,
            tc.tile_pool(name="ps_o", bufs=1, space="PSUM") as pso,
        ):
            # ---------------- load weights / constants ----------------
            qkvw = pp.tile([P, CCH, 3 * C], BF16, tag="qkvw")
            nc.sync.dma_start(qkvw[:], qkvw_d.rearrange("t p o -> p t o"))
            projw = pp.tile([P, CCH, C], BF16, tag="projw")
            nc.sync.dma_start(projw[:], projw_d.rearrange("t p o -> p t o"))
            qkb = pp.tile([P, 4], F32, tag="qkb")
            nc.sync.dma_start(qkb[:], qkbias_d.rearrange("j p one -> p (j one)"))
            projb = pp.tile([P, CCH], F32, tag="projb")
            nc.sync.dma_start(projb[:], projb_d.rearrange("t p one -> p (t one)"))
            gnsc = pp.tile([P, CCH], F32, tag="gnsc")
            nc.sync.dma_start(gnsc[:], gnsc_d.rearrange("t p one -> p (t one)"))
            gnbi = pp.tile([P, CCH], F32, tag="gnbi")
            nc.sync.dma_start(gnbi[:], gnbi_d.rearrange("t p one -> p (t one)"))
            gnind = pp.tile([P, CCH, P], F32, tag="gnind")
            nc.sync.dma_start(gnind[:], gnind_d.rearrange("t p g -> p t g"))
            gnind2 = pp.tile([P, CCH, P], F32, tag="gnind2")
            nc.sync.dma_start(gnind2[:], gnind2_d.rearrange("t g c -> g t c"))
            # V bias broadcast across partitions (DMA with partition-stride 0)
            vbias = pp.tile([P, C], F32, tag="vbias")
            nc.sync.dma_start(vbias[:], vbias_d[None, :].to_broadcast([P, C]))
            # all-ones [128, 128] used to (sum over partitions + broadcast)
            ones_f32 = pp.tile([P, P], F32, tag="ones_f32")
            nc.vector.memset(ones_f32[:], 1.0)

            # ---------------- load x, GroupNorm stats ----------------
            x_sb = pp.tile([P, CCH, n_tok], F32, tag="x_sb")
            stats = pp.tile([P, CCH, 2], F32, tag="stats")
            XPC = max(1, n_tok // 1024)
            for t in range(CCH):
                for pc in range(XPC):
                    xs = slice(pc * (n_tok // XPC), (pc + 1) * (n_tok // XPC))
                    nc.sync.dma_start(x_sb[:, t, xs], x_d[t * P:(t + 1) * P, xs])
                bn6 = wp.tile([P, n_tok // 512, 6], F32, tag="bn6")
                xv = x_sb[:, t].rearrange("p (a b) -> p a b", b=512)
                for a in range(n_tok // 512):
                    nc.vector.bn_stats(bn6[:, a], xv[:, a])
                # mv = (mean, var) per partition
                nc.vector.bn_aggr(stats[:, t], bn6[:])
                # stats col1 := mean^2 + var = E[x^2] (col0 stays mean)
                nc.vector.scalar_tensor_tensor(
                    out=stats[:, t, 1:2],
                    in0=stats[:, t, 0:1],
                    scalar=stats[:, t, 0:1],
                    in1=stats[:, t, 1:2],
                    op0=ALU.mult,
                    op1=ALU.add,
                )

            # group aggregation: gagg[g, j] = sum_{c in group g} stats[c, j]
            gagg_ps = psb.tile([P, 2, 512], F32, tag="sc", name="gagg_ps")
            for t in range(CCH):
                nc.tensor.matmul(
                    gagg_ps[:, 0, :2],
                    gnind[:, t],
                    stats[:, t],
                    start=(t == 0),
                    stop=(t == CCH - 1),
                )
            # per-group a = rstd, b = -mean * rstd   (divide sums by GSZ first)
            gab = pp.tile([P, 2], F32, tag="gab")
            nc.vector.memset(gab[:], 0.0)
            gmean = wp.tile([P, 1], F32, tag="gmean")
            gtmp = wp.tile([P, 1], F32, tag="gtmp")
            nc.vector.tensor_scalar_mul(gmean[:GROUPS], gagg_ps[:GROUPS, 0, 0:1], 1.0 / GSZ)
            nc.vector.tensor_scalar_mul(gtmp[:GROUPS], gagg_ps[:GROUPS, 0, 1:2], 1.0 / GSZ)
            # gtmp := mean^2 - E[x^2] = -var
            nc.vector.scalar_tensor_tensor(
                out=gtmp[:GROUPS],
                in0=gmean[:GROUPS],
                scalar=gmean[:GROUPS],
                in1=gtmp[:GROUPS],
                op0=ALU.mult,
                op1=ALU.subtract,
            )
            # std = sqrt(-1 * gtmp + eps)
            epsb = wp.tile([P, 1], F32, tag="epsb")
            nc.vector.memset(epsb[:], EPS)
            nc.scalar.activation(gtmp[:GROUPS], gtmp[:GROUPS], AF.Sqrt,
                                 bias=epsb[:GROUPS], scale=-1.0)
            nc.vector.reciprocal(gab[:GROUPS, 0:1], gtmp[:GROUPS])  # a = rstd
            # b = -(mean * rstd):  (mean mult rstd) subtract 2*mean*rstd
            nc.vector.tensor_mul(gtmp[:GROUPS], gmean[:GROUPS], gab[:GROUPS, 0:1])
            nc.vector.tensor_scalar_mul(gab[:GROUPS, 1:2], gtmp[:GROUPS], -1.0)

            # broadcast (a, b) back to channels: chab[c, j] = gab[g(c), j]
            xn = pp.tile([P, CCH, n_tok], BF16, tag="xn")
            for t in range(CCH):
                chab_ps = psb.tile([P, 2, 512], F32, tag="sc", name=f"chab_ps{t}")[:, 0]
                nc.tensor.matmul(chab_ps[:, :2], gnind2[:, t], gab[:],
                                 start=True, stop=True)
                # mult_c = a * gamma_c ; add_c = b * gamma_c + beta_c
                chm = pp.tile([P, 1], F32, tag=f"chm{t}", name=f"chm{t}")
                cha = pp.tile([P, 1], F32, tag=f"cha{t}", name=f"cha{t}")
                nc.vector.tensor_mul(chm[:], chab_ps[:, 0:1], gnsc[:, t, None])
                nc.vector.scalar_tensor_tensor(
                    out=cha[:],
                    in0=chab_ps[:, 1:2],
                    scalar=gnsc[:, t, None],
                    in1=gnbi[:, t, None],
                    op0=ALU.mult,
                    op1=ALU.add,
                )
                # xn = x * mult + add   (bf16 out)
                nc.scalar.activation(xn[:, t], x_sb[:, t], AF.Identity,
                                     bias=cha[:], scale=chm[:])

            # ---------------- QKV ----------------
            # Q, K in (d, n) layout; j = 0,1 -> Q chunks; 2,3 -> K chunks.
            # softmax scale folded into Q (scale on PSUM->SBUF copy; host
            # pre-scaled the Q bias entries). N=1024 matmuls with bf16 PSUM
            # halve the matmul/copy instruction counts.
            # Q, K, V stored in fp8e4 (e4m3) for DoubleRow matmuls. The
            # softmax scale is NOT folded into Q here (it would push |q| to
            # ~0.02, into fp8 subnormals) — it moves into the exp() scale.
            qk = pp.tile([P, 4, n_tok], FP8, tag="qk")
            for j in range(4):
                for np2 in range(NQ // 2):
                    ns = slice(np2 * QP, (np2 + 1) * QP)
                    ps = psb.tile([P, 2, 512], F32, tag="sc",
                                  name=f"qk{j}_{np2}")
                    for half in range(2):
                        nsh = slice(np2 * QP + half * QT,
                                    np2 * QP + (half + 1) * QT)
                        for t in range(CCH):
                            nc.tensor.matmul(
                                ps[:, half],
                                qkvw[:, t, j * P:(j + 1) * P],
                                xn[:, t, nsh],
                                start=(t == 0),
                                stop=(t == CCH - 1),
                            )
                    nc.scalar.activation(
                        qk[:, j, ns], ps.rearrange("p a b -> p (a b)"),
                        AF.Identity,
                        bias=qkb[:, j, None],
                        scale=1.0,
                    )
            # V token-major: v_sb[:, kb, d] holds V[token kb*128+p, d]
            v_sb = pp.tile([P, NKB, C], FP8, tag="v_sb")
            for kbp in range(NKB // 2):
                ps = psb.tile([P, 2, 512], F32, tag="sc", name=f"v{kbp}")
                for k2 in range(2):
                    for t in range(CCH):
                        nc.tensor.matmul(
                            ps[:, k2, :C],
                            xn[:, t, (2 * kbp + k2) * P:(2 * kbp + k2 + 1) * P],
                            qkvw[:, t, 2 * C:3 * C],
                            start=(t == 0),
                            stop=(t == CCH - 1),
                        )
                nc.vector.tensor_add(
                    v_sb[:, 2 * kbp:2 * kbp + 2],
                    ps[:, :, :C],
                    vbias[:, None, :].to_broadcast([P, 2, C]),
                )

            # ---------------- attention + proj + residual ----------------
            # Processed in q-tile PAIRS (1024 q columns): scores/exp/s_acc
            # run pair-wide; PV stays per-qt (PSUM fp32 accumulation).
            # Each pair's finalize (denominator, normalize, proj, residual)
            # is DEFERRED into the next pair's kb loop so the PE never
            # stalls on the DVE chain at pair boundaries.
            def finalize_pair(st):
                pr, o_ps, s_accA, s_accB = st
                rec = wp.tile([P, QP], F32, tag="rec", bufs=2)
                obs = []
                for qi in range(2):
                    qh = slice(qi * QT, (qi + 1) * QT)
                    den = psb.tile([P, 2, 512], F32, tag="sc",
                                   name=f"den_{pr}_{qi}")[:, 0]
                    nc.tensor.matmul(den[:], ones_f32[:], s_accB[:, qh],
                                     start=True, stop=False)
                    nc.tensor.matmul(den[:], ones_f32[:], s_accA[:, qh],
                                     start=False, stop=True)
                    nc.vector.reciprocal_approx_fast(rec[:, qh], den[:])
                for ch in range(2):
                    ob = wp.tile([P, QP], BF16, tag=f"ob{ch}", bufs=2,
                                 name=f"ob{ch}_{pr}")
                    for qi in range(2):
                        qh = slice(qi * QT, (qi + 1) * QT)
                        nc.vector.tensor_mul(ob[:, qh], o_ps[qi][ch][:],
                                             rec[:, qh])
                    obs.append(ob)
                for t in range(CCH):
                    for qi in range(2):
                        qh = slice(qi * QT, (qi + 1) * QT)
                        qg = slice(pr * QP + qi * QT, pr * QP + (qi + 1) * QT)
                        p_ps = psb.tile([P, 512], F32, tag="sc",
                                        name=f"p_{pr}_{t}_{qi}")
                        nc.tensor.matmul(p_ps[:],
                                         projw[:, 0, t * P:(t + 1) * P],
                                         obs[0][:, qh], start=True, stop=False)
                        nc.tensor.matmul(p_ps[:],
                                         projw[:, 1, t * P:(t + 1) * P],
                                         obs[1][:, qh], start=False, stop=True)
                        res = wp.tile([P, QT], F32, tag="res", bufs=3)
                        nc.vector.scalar_tensor_tensor(
                            out=res[:],
                            in0=p_ps[:],
                            scalar=projb[:, t, None],
                            in1=x_sb[:, t, qg],
                            op0=ALU.add,
                            op1=ALU.add,
                        )
                        nc.sync.dma_start(out_d[t * P:(t + 1) * P, qg], res[:])

            pending = None
            for pr in range(NQP):
                qps = slice(pr * QP, (pr + 1) * QP)
                o_ps = [[pso.tile([P, 512], F32, tag=f"o{qi}{ch}",
                                  name=f"o{qi}{ch}_{pr}")
                         for ch in range(2)] for qi in range(2)]
                # two running denominator accumulators: even key blocks on
                # the vector engine, odd ones on the (otherwise idle) gpsimd
                # engine; combined (in bf16) at finalize time.
                s_accA = wp.tile([P, QP], F32, tag="s_accA", bufs=2)
                s_accB = wp.tile([P, QP], F32, tag="s_accB", bufs=2)
                for kbp in range(NKB // 2):
                    # pt holds exp(scores) for the TWO key blocks of this
                    # DoubleRow pair: plane i = key block kbp*2+i (fp8).
                    pt = wp.tile([P, 2, QP], FP8, tag="pt", bufs=6)
                    for k2 in range(2):
                        kb = 2 * kbp + k2
                        s_ps = psb.tile([P, 2, 512], F32, tag="sc",
                                        name=f"s_{pr}_{kb}")
                        for qi in range(2):
                            qh = slice(pr * QP + qi * QT,
                                       pr * QP + (qi + 1) * QT)
                            # scores via one DoubleRow matmul: contraction
                            # over all 256 channels (two 128-planes).
                            nc.tensor.matmul(
                                s_ps[:, qi],
                                qk[:, 2:4, kb * P:(kb + 1) * P],
                                qk[:, 0:2, qh],
                                start=True,
                                stop=True,
                                perf_mode=DR,
                            )
                        nc.scalar.activation(
                            pt[:, k2],
                            s_ps.rearrange("p a b -> p (a b)"),
                            AF.Exp, scale=ATT_SCALE)
                        # denominator accumulation: the gpsimd add is ~2x the
                        # DVE's, so gpsimd only takes the odd-plane add on
                        # every other kbp (plus the last, so the DVE is free
                        # at the pair boundary) — keeps both chains well
                        # inside the PE's per-kbp budget instead of letting
                        # gpsimd set the loop cadence.
                        on_gp = (kbp % 2 == 0) and kbp != NKB // 2 - 1
                        if kb == 0:
                            nc.vector.tensor_copy(s_accA[:], pt[:, 0])
                        elif kb == 1:
                            nc.gpsimd.tensor_copy(s_accB[:], pt[:, 1])
                        elif k2 == 0:
                            nc.vector.tensor_add(s_accA[:], s_accA[:],
                                                 pt[:, 0])
                        elif on_gp:
                            nc.gpsimd.tensor_add(s_accB[:], s_accB[:],
                                                 pt[:, 1])
                        else:
                            nc.vector.tensor_add(s_accA[:], s_accA[:],
                                                 pt[:, 1])
                    for ch in range(2):
                        vt = v_sb[:, 2 * kbp:2 * kbp + 2,
                                  ch * P:(ch + 1) * P]
                        for qi in range(2):
                            nc.tensor.matmul(
                                o_ps[qi][ch][:], vt,
                                pt[:, :, qi * QT:(qi + 1) * QT],
                                start=(kbp == 0), stop=(kbp == NKB // 2 - 1),
                                perf_mode=DR)
                    if kbp == 1 and pending is not None:
                        finalize_pair(pending)
                        pending = None
                pending = (pr, o_ps, s_accA, s_accB)
            finalize_pair(pending)

    nc.finalize()
    return nc


# ---------------------------------------------------------------------------
# host side
# ---------------------------------------------------------------------------

def _prep_core_inputs(inputs, n_tok=H * W):
    """Build the per-core in_maps (shared weight tensors + per-core x)."""
    CCH = C // P
    f32 = np.float32
    bf16 = ml_dtypes.bfloat16

    x = np.asarray(inputs["x"], f32).reshape(B, C, n_tok)
    gn_scale = np.asarray(inputs["gn_scale"], f32)
    gn_bias = np.asarray(inputs["gn_bias"], f32)
    qkv_w = np.asarray(inputs["qkv_w"], f32)
    qkv_b = np.asarray(inputs["qkv_b"], f32)
    proj_w = np.asarray(inputs["proj_w"], f32)
    proj_b = np.asarray(inputs["proj_b"], f32)

    qkv_wt = np.ascontiguousarray(qkv_w.T).reshape(CCH, P, 3 * C).astype(bf16)
    qk_bias = qkv_b[:2 * C].reshape(4, P, 1).astype(f32).copy()
    v_bias = qkv_b[2 * C:].astype(f32)
    proj_wt = np.ascontiguousarray(proj_w.T).reshape(CCH, P, C).astype(bf16)
    proj_bt = proj_b.reshape(CCH, P, 1).astype(f32)
    gn_sc = gn_scale.reshape(CCH, P, 1).astype(f32)
    gn_bi = gn_bias.reshape(CCH, P, 1).astype(f32)

    ch = np.arange(C)
    gn_ind = np.zeros((CCH, P, P), f32)
    gn_ind[ch // P, ch % P, ch // (C // GROUPS)] = 1.0
    gn_ind2 = np.zeros((CCH, P, P), f32)
    for t in range(CCH):
        gn_ind2[t, :GROUPS, :] = gn_ind[t, :, :GROUPS].T

    shared = {
        "qkv_wt": qkv_wt,
        "qk_bias": qk_bias,
        "v_bias": v_bias,
        "proj_wt": proj_wt,
        "proj_b": proj_bt,
        "gn_sc": gn_sc,
        "gn_bi": gn_bi,
        "gn_ind": gn_ind,
        "gn_ind2": gn_ind2,
    }
    return [dict(shared, x=np.ascontiguousarray(x[i])) for i in range(B)]


_NC_CACHE = {}
LAST_RESULT = None  # BassKernelResults of the most recent run (for test.py)


def _get_nc():
    if "nc" not in _NC_CACHE:
        _NC_CACHE["nc"] = build_nc()
    return _NC_CACHE["nc"]


def kernel(**inputs) -> np.ndarray:
    global LAST_RESULT
    from concourse.bass_utils import run_bass_kernel_spmd

    nc = _get_nc()
    in_maps = _prep_core_inputs(inputs)
    res = run_bass_kernel_spmd(nc, in_maps, list(range(N_CORES)))
    LAST_RESULT = res
    out = np.stack([np.asarray(res.results[i]["out"]) for i in range(B)])
    return out.reshape(B, C, H, W).astype(np.float32)



# revision 8
# speedup vs baseline: 1.2308x; 1.2308x over previous
"""Trainium2 Bass kernel for nn_AttentionBlock (GroupNorm + single-head
self-attention + proj + residual), data-parallel over batch on 8 cores.

Contract: kernel(**inputs) takes the FULL unsharded inputs
  x (8, 256, 64, 64) f32, gn_scale (256,), gn_bias (256,),
  qkv_w (768, 256), qkv_b (768,), proj_w (256, 256), proj_b (256,)
and returns the FULL output (8, 256, 64, 64) f32.

Per-core plan (one sample per core):
  - x viewed as (C=256, N=4096) = (channels on partitions, tokens on free dim)
  - GroupNorm(8 groups) stats via bn_stats/bn_aggr + tiny indicator matmuls
  - xn cast to bf16; QKV as channel matmuls:
      Q, K produced in (d, n) layout  [d on partitions], stored fp8e4
      V produced in token-major (n, d) layout, stored fp8e4
  - attention runs over q-tiles of 512 columns x key-block PAIRS (2x128
    keys -> one DoubleRow contraction):
      scoresT[k, q] = sum_d K[d,k] Q[d,q]     (one DR matmul per kb)
      PT = exp(scoresT * scale)               (one ACT instr per kb-pair,
                                               [128, 1024] granularity)
      den[q]  += ones[k]^T PT                 (M=1 DR matmul, accumulated in
                                               a PSUM row across the q-tile;
                                               rows 0/32/64/96 by qt%4)
      o_un[d, q] += V[k,d]^T PT               (2 DR matmuls, f32 PSUM)
    -> NO per-step DVE/gpsimd work at all; the whole denominator rides on
    the PE (cost ~= 1 extra N=512 matmul per kb-pair).
  - finalize per q-tile (deferred into the next q-tile's first 2 steps):
      ob = bf16(o_un)                  (DVE copies; unblocks next tile's PV)
      rec = 1/den  -> DMA broadcast to 128 partitions via a DRAM bounce
      p = proj(ob); out = p*rec + proj_b + x   (normalize AFTER proj --
                                                rec commutes with the matmul)
PSUM budget (8 banks): scores [P,2,512]f32 x2bufs = 4, o_un x2 = 2,
den row bank = 1, proj psum = 1.
"""

import os
import sys

import numpy as np

for _p in (
    "/opt/trn_rl_repo",
    "/root/.axon_site",
    "/root/.axon_site/_ro/trn_rl_repo",
    "/root/.axon_site/_ro/pypackages",
):
    if os.path.isdir(_p) and _p not in sys.path:
        sys.path.append(_p)

import ml_dtypes  # noqa: E402

import concourse.bass as bass  # noqa: E402
import concourse.mybir as mybir  # noqa: E402
import concourse.tile as tile  # noqa: E402
from concourse import bacc  # noqa: E402

F32 = mybir.dt.float32
BF16 = mybir.dt.bfloat16
FP8 = mybir.dt.float8e4
AF = mybir.ActivationFunctionType
ALU = mybir.AluOpType
DR = mybir.MatmulPerfMode.DoubleRow

B, C, H, W = 8, 256, 64, 64
GROUPS = 8
EPS = 1e-5
P = 128
N_CORES = 8
ATT_SCALE = float(C) ** -0.5  # 1/16


def build_nc(n_tok=H * W):
    """Build the single-core Bass program (SPMD across 8 cores)."""
    CCH = C // P            # channel chunks (2)
    QT = 512                # q-tile width (one PSUM bank of f32)
    NQT = n_tok // QT       # number of q tiles (8)
    NKB = n_tok // P        # number of 128-token key blocks (32)
    NS = NKB // 2           # key-block PAIRS per q-tile (16)
    GSZ = C // GROUPS       # channels per group (32)

    nc = bacc.Bacc()

    # ---- DRAM I/O (per-core tensors; host shards batch over cores) ----
    x_d = nc.dram_tensor("x", [C, n_tok], F32, kind="ExternalInput")
    qkvw_d = nc.dram_tensor("qkv_wt", [CCH, P, 3 * C], BF16, kind="ExternalInput")
    qkbias_d = nc.dram_tensor("qk_bias", [4, P, 1], F32, kind="ExternalInput")
    vbias_d = nc.dram_tensor("v_bias", [C], F32, kind="ExternalInput")
    projw_d = nc.dram_tensor("proj_wt", [CCH, P, C], BF16, kind="ExternalInput")
    projb_d = nc.dram_tensor("proj_b", [CCH, P, 1], F32, kind="ExternalInput")
    gnsc_d = nc.dram_tensor("gn_sc", [CCH, P, 1], F32, kind="ExternalInput")
    gnbi_d = nc.dram_tensor("gn_bi", [CCH, P, 1], F32, kind="ExternalInput")
    # group-sum indicator (zero-padded to M=128 so the matmul avoids the
    # 32-column tile-mode lowering): ind[t, c, g] = (t*128 + c) // 32 == g
    gnind_d = nc.dram_tensor("gn_ind", [CCH, P, P], F32, kind="ExternalInput")
    # channel-broadcast indicator, padded to K=128: ind2[t, g, c] nonzero only g<8
    gnind2_d = nc.dram_tensor("gn_ind2", [CCH, P, P], F32, kind="ExternalInput")
    out_d = nc.dram_tensor("out", [C, n_tok], F32, kind="ExternalOutput")

    with tile.TileContext(nc) as tc:
        with (
            tc.tile_pool(name="persist", bufs=1) as pp,
            tc.tile_pool(name="work", bufs=3) as wp,
            tc.tile_pool(name="ps_sc", bufs=2, space="PSUM") as psb,
            tc.tile_pool(name="ps_o", bufs=1, space="PSUM") as pso,
            tc.tile_pool(name="ps_den", bufs=1, space="PSUM") as psd,
        ):
            # ---------------- load weights / constants ----------------
            qkvw = pp.tile([P, CCH, 3 * C], BF16, tag="qkvw")
            nc.sync.dma_start(qkvw[:], qkvw_d.rearrange("t p o -> p t o"))
            projw = pp.tile([P, CCH, C], BF16, tag="projw")
            nc.sync.dma_start(projw[:], projw_d.rearrange("t p o -> p t o"))
            qkb = pp.tile([P, 4], F32, tag="qkb")
            nc.sync.dma_start(qkb[:], qkbias_d.rearrange("j p one -> p (j one)"))
            projb = pp.tile([P, CCH], F32, tag="projb")
            nc.sync.dma_start(projb[:], projb_d.rearrange("t p one -> p (t one)"))
            gnsc = pp.tile([P, CCH], F32, tag="gnsc")
            nc.sync.dma_start(gnsc[:], gnsc_d.rearrange("t p one -> p (t one)"))
            gnbi = pp.tile([P, CCH], F32, tag="gnbi")
            nc.sync.dma_start(gnbi[:], gnbi_d.rearrange("t p one -> p (t one)"))
            gnind = pp.tile([P, CCH, P], F32, tag="gnind")
            nc.sync.dma_start(gnind[:], gnind_d.rearrange("t p g -> p t g"))
            gnind2 = pp.tile([P, CCH, P], F32, tag="gnind2")
            nc.sync.dma_start(gnind2[:], gnind2_d.rearrange("t g c -> g t c"))
            # V bias broadcast across partitions (DMA with partition-stride 0)
            vbias = pp.tile([P, C], F32, tag="vbias")
            nc.sync.dma_start(vbias[:], vbias_d[None, :].to_broadcast([P, C]))
            # fp8 all-ones block: lhsT of the denominator matmuls (M=128 so
            # every PSUM partition gets the column sum -> broadcast for free;
            # M<128 would trigger the 32-column tile-mode lowering, which
            # crashes the exec unit)
            ones8 = pp.tile([P, 2, P], FP8, tag="ones8")
            nc.vector.memset(ones8[:], 1.0)

            # ---------------- load x, GroupNorm stats ----------------
            x_sb = pp.tile([P, CCH, n_tok], F32, tag="x_sb")
            stats = pp.tile([P, CCH, 2], F32, tag="stats")
            XPC = max(1, n_tok // 1024)
            for t in range(CCH):
                for pc in range(XPC):
                    xs = slice(pc * (n_tok // XPC), (pc + 1) * (n_tok // XPC))
                    nc.sync.dma_start(x_sb[:, t, xs], x_d[t * P:(t + 1) * P, xs])
                bn6 = wp.tile([P, n_tok // 512, 6], F32, tag="bn6")
                xv = x_sb[:, t].rearrange("p (a b) -> p a b", b=512)
                for a in range(n_tok // 512):
                    nc.vector.bn_stats(bn6[:, a], xv[:, a])
                # mv = (mean, var) per partition
                nc.vector.bn_aggr(stats[:, t], bn6[:])
                # stats col1 := mean^2 + var = E[x^2] (col0 stays mean)
                nc.vector.scalar_tensor_tensor(
                    out=stats[:, t, 1:2],
                    in0=stats[:, t, 0:1],
                    scalar=stats[:, t, 0:1],
                    in1=stats[:, t, 1:2],
                    op0=ALU.mult,
                    op1=ALU.add,
                )

            # group aggregation: gagg[g, j] = sum_{c in group g} stats[c, j]
            gagg_ps = psb.tile([P, 2, 512], F32, tag="sc", name="gagg_ps")
            for t in range(CCH):
                nc.tensor.matmul(
                    gagg_ps[:, 0, :2],
                    gnind[:, t],
                    stats[:, t],
                    start=(t == 0),
                    stop=(t == CCH - 1),
                )
            # per-group a = rstd, b = -mean * rstd   (divide sums by GSZ first)
            gab = pp.tile([P, 2], F32, tag="gab")
            nc.vector.memset(gab[:], 0.0)
            gmean = wp.tile([P, 1], F32, tag="gmean")
            gtmp = wp.tile([P, 1], F32, tag="gtmp")
            nc.vector.tensor_scalar_mul(gmean[:GROUPS], gagg_ps[:GROUPS, 0, 0:1], 1.0 / GSZ)
            nc.vector.tensor_scalar_mul(gtmp[:GROUPS], gagg_ps[:GROUPS, 0, 1:2], 1.0 / GSZ)
            # gtmp := mean^2 - E[x^2] = -var
            nc.vector.scalar_tensor_tensor(
                out=gtmp[:GROUPS],
                in0=gmean[:GROUPS],
                scalar=gmean[:GROUPS],
                in1=gtmp[:GROUPS],
                op0=ALU.mult,
                op1=ALU.subtract,
            )
            # std = sqrt(-1 * gtmp + eps)
            epsb = wp.tile([P, 1], F32, tag="epsb")
            nc.vector.memset(epsb[:], EPS)
            nc.scalar.activation(gtmp[:GROUPS], gtmp[:GROUPS], AF.Sqrt,
                                 bias=epsb[:GROUPS], scale=-1.0)
            nc.vector.reciprocal(gab[:GROUPS, 0:1], gtmp[:GROUPS])  # a = rstd
            # b = -(mean * rstd)
            nc.vector.tensor_mul(gtmp[:GROUPS], gmean[:GROUPS], gab[:GROUPS, 0:1])
            nc.vector.tensor_scalar_mul(gab[:GROUPS, 1:2], gtmp[:GROUPS], -1.0)

            # broadcast (a, b) back to channels: chab[c, j] = gab[g(c), j]
            xn = pp.tile([P, CCH, n_tok], BF16, tag="xn")
            for t in range(CCH):
                chab_ps = psb.tile([P, 2, 512], F32, tag="sc", name=f"chab_ps{t}")[:, 0]
                nc.tensor.matmul(chab_ps[:, :2], gnind2[:, t], gab[:],
                                 start=True, stop=True)
                # mult_c = a * gamma_c ; add_c = b * gamma_c + beta_c
                chm = pp.tile([P, 1], F32, tag=f"chm{t}", name=f"chm{t}")
                cha = pp.tile([P, 1], F32, tag=f"cha{t}", name=f"cha{t}")
                nc.vector.tensor_mul(chm[:], chab_ps[:, 0:1], gnsc[:, t, None])
                nc.vector.scalar_tensor_tensor(
                    out=cha[:],
                    in0=chab_ps[:, 1:2],
                    scalar=gnsc[:, t, None],
                    in1=gnbi[:, t, None],
                    op0=ALU.mult,
                    op1=ALU.add,
                )
                # xn = x * mult + add   (bf16 out); t=0 on ACT, t=1 on DVE
                if t == 0:
                    nc.scalar.activation(xn[:, t], x_sb[:, t], AF.Identity,
                                         bias=cha[:], scale=chm[:])
                else:
                    nc.vector.tensor_scalar(xn[:, t], x_sb[:, t], chm[:], cha[:],
                                            op0=ALU.mult, op1=ALU.add)

            # ---------------- QKV ----------------
            # Q, K in (d, n) layout; j = 0,1 -> Q chunks; 2,3 -> K chunks.
            # Stored fp8e4 for DoubleRow matmuls; softmax scale stays in the
            # exp() (folding it into Q would push |q| into fp8 subnormals).
            # PSUM->SBUF copies alternate ACT / DVE to halve the copy wall.
            qk = pp.tile([P, 4, n_tok], FP8, tag="qk")
            for j in range(4):
                for np2 in range(NQT // 2):
                    ns = slice(np2 * 2 * QT, (np2 + 1) * 2 * QT)
                    ps = psb.tile([P, 2, 512], F32, tag="sc",
                                  name=f"qk{j}_{np2}")
                    for half in range(2):
                        nsh = slice(np2 * 2 * QT + half * QT,
                                    np2 * 2 * QT + (half + 1) * QT)
                        for t in range(CCH):
                            nc.tensor.matmul(
                                ps[:, half],
                                qkvw[:, t, j * P:(j + 1) * P],
                                xn[:, t, nsh],
                                start=(t == 0),
                                stop=(t == CCH - 1),
                            )
                    if (j * (NQT // 2) + np2) % 2 == 0:
                        nc.scalar.activation(
                            qk[:, j, ns], ps.rearrange("p a b -> p (a b)"),
                            AF.Identity,
                            bias=qkb[:, j, None],
                            scale=1.0,
                        )
                    else:
                        nc.vector.tensor_scalar_add(
                            qk[:, j, ns], ps.rearrange("p a b -> p (a b)"),
                            qkb[:, j, None],
                        )
            # V token-major: v_sb[:, kb, d] holds V[token kb*128+p, d]
            v_sb = pp.tile([P, NKB, C], FP8, tag="v_sb")
            for kbp in range(NKB // 2):
                ps = psb.tile([P, 2, 512], F32, tag="sc", name=f"v{kbp}")
                for k2 in range(2):
                    for t in range(CCH):
                        nc.tensor.matmul(
                            ps[:, k2, :C],
                            xn[:, t, (2 * kbp + k2) * P:(2 * kbp + k2 + 1) * P],
                            qkvw[:, t, 2 * C:3 * C],
                            start=(t == 0),
                            stop=(t == CCH - 1),
                        )
                # NB: gpsimd has no PSUM port -- these reads must stay on DVE
                nc.vector.tensor_add(
                    v_sb[:, 2 * kbp:2 * kbp + 2],
                    ps[:, :, :C],
                    vbias[:, None, :].to_broadcast([P, 2, C]),
                )

            # ---------------- attention + proj + residual ----------------
            # q-tiles of 512 columns; per step = one key-block PAIR.
            # den accumulates on the PE into one PSUM row (qt%4 -> 0/32/64/96).
            den_all = psd.tile([P, QT], F32, tag="den", name="den_all")

            def emit_den_pv(pt, s, o_ps, qt):
                nc.tensor.matmul(den_all[:], ones8[:], pt[:],
                                 start=(s == 0), stop=(s == NS - 1),
                                 perf_mode=DR)
                for ch in range(2):
                    nc.tensor.matmul(
                        o_ps[ch][:],
                        v_sb[:, 2 * s:2 * s + 2, ch * P:(ch + 1) * P],
                        pt[:],
                        start=(s == 0), stop=(s == NS - 1), perf_mode=DR)

            def fin_stage1(qt, o_ps):
                # recip first (releases the den bank for the next q-tile's
                # den matmul); then the o_un bf16 copies (DVE: gpsimd has no
                # PSUM port) release the o_ps banks for the next tile's PV.
                rec_bc = wp.tile([P, QT], F32, tag="rec_bc", bufs=2,
                                 name=f"rec_bc_{qt}")
                nc.vector.reciprocal_approx_fast(rec_bc[:], den_all[:])
                obs = []
                for ch in range(2):
                    ob = wp.tile([P, QT], BF16, tag=f"ob{ch}", bufs=2,
                                 name=f"ob{ch}_{qt}")
                    nc.vector.tensor_copy(ob[:], o_ps[ch][:])
                    obs.append(ob)
                return obs, rec_bc

            def fin_stage2(qt, obs, rec_bc):
                qs = slice(qt * QT, (qt + 1) * QT)
                for t in range(CCH):
                    p_ps = psb.tile([P, QT], F32, tag="pp", bufs=1,
                                    name=f"p_{qt}_{t}")
                    nc.tensor.matmul(p_ps[:], projw[:, 0, t * P:(t + 1) * P],
                                     obs[0][:], start=True, stop=False)
                    nc.tensor.matmul(p_ps[:], projw[:, 1, t * P:(t + 1) * P],
                                     obs[1][:], start=False, stop=True)
                    # out = p*rec + proj_b + x  (normalize after proj)
                    tmp = wp.tile([P, QT], F32, tag="tmp", bufs=2)
                    nc.vector.tensor_mul(tmp[:], p_ps[:], rec_bc[:])
                    res = wp.tile([P, QT], F32, tag="res", bufs=3)
                    nc.vector.scalar_tensor_tensor(
                        out=res[:],
                        in0=tmp[:],
                        scalar=projb[:, t, None],
                        in1=x_sb[:, t, qs],
                        op0=ALU.add,
                        op1=ALU.add,
                    )
                    nc.sync.dma_start(out_d[t * P:(t + 1) * P, qs], res[:])

            prev = None       # (pt, s, o_ps, qt) awaiting den+PV emission
            pending = None    # (qt, o_ps) awaiting finalize
            hold = None       # (obs, rec_bc) between fin stages
            for qt in range(NQT):
                qs = slice(qt * QT, (qt + 1) * QT)
                o_ps = [pso.tile([P, QT], F32, tag=f"o{ch}",
                                 name=f"o{ch}_{qt}") for ch in range(2)]
                for s in range(NS):
                    pt = wp.tile([P, 2, QT], FP8, tag="pt", bufs=4)
                    s_ps = psb.tile([P, 2, QT], F32, tag="sc",
                                    name=f"s_{qt}_{s}")
                    for k2 in range(2):
                        kb = 2 * s + k2
                        nc.tensor.matmul(
                            s_ps[:, k2],
                            qk[:, 2:4, kb * P:(kb + 1) * P],
                            qk[:, 0:2, qs],
                            start=True, stop=True, perf_mode=DR)
                    nc.scalar.activation(
                        pt.rearrange("p a b -> p (a b)"),
                        s_ps.rearrange("p a b -> p (a b)"),
                        AF.Exp, scale=ATT_SCALE)
                    if prev is not None:
                        emit_den_pv(*prev)
                    prev = (pt, s, o_ps, qt)
                    if s == 0 and pending is not None:
                        hold = fin_stage1(*pending)
                    elif s == 1 and pending is not None:
                        fin_stage2(pending[0], *hold)
                        pending = None
                        hold = None
                pending = (qt, o_ps)
            emit_den_pv(*prev)
            hold = fin_stage1(*pending)
            fin_stage2(pending[0], *hold)

    nc.finalize()
    return nc


# ---------------------------------------------------------------------------
# host side
# ---------------------------------------------------------------------------

def _prep_core_inputs(inputs, n_tok=H * W):
    """Build the per-core in_maps (shared weight tensors + per-core x)."""
    CCH = C // P
    f32 = np.float32
    bf16 = ml_dtypes.bfloat16

    x = np.asarray(inputs["x"], f32).reshape(B, C, n_tok)
    gn_scale = np.asarray(inputs["gn_scale"], f32)
    gn_bias = np.asarray(inputs["gn_bias"], f32)
    qkv_w = np.asarray(inputs["qkv_w"], f32)
    qkv_b = np.asarray(inputs["qkv_b"], f32)
    proj_w = np.asarray(inputs["proj_w"], f32)
    proj_b = np.asarray(inputs["proj_b"], f32)

    qkv_wt = np.ascontiguousarray(qkv_w.T).reshape(CCH, P, 3 * C).astype(bf16)
    qk_bias = qkv_b[:2 * C].reshape(4, P, 1).astype(f32).copy()
    v_bias = qkv_b[2 * C:].astype(f32)
    proj_wt = np.ascontiguousarray(proj_w.T).reshape(CCH, P, C).astype(bf16)
    proj_bt = proj_b.reshape(CCH, P, 1).astype(f32)
    gn_sc = gn_scale.reshape(CCH, P, 1).astype(f32)
    gn_bi = gn_bias.reshape(CCH, P, 1).astype(f32)

    ch = np.arange(C)
    gn_ind = np.zeros((CCH, P, P), f32)
    gn_ind[ch // P, ch % P, ch // (C // GROUPS)] = 1.0
    gn_ind2 = np.zeros((CCH, P, P), f32)
    for t in range(CCH):
        gn_ind2[t, :GROUPS, :] = gn_ind[t, :, :GROUPS].T

    shared = {
        "qkv_wt": qkv_wt,
        "qk_bias": qk_bias,
        "v_bias": v_bias,
        "proj_wt": proj_wt,
        "proj_b": proj_bt,
        "gn_sc": gn_sc,
        "gn_bi": gn_bi,
        "gn_ind": gn_ind,
        "gn_ind2": gn_ind2,
    }
    return [dict(shared, x=np.ascontiguousarray(x[i])) for i in range(B)]


_NC_CACHE = {}
LAST_RESULT = None  # BassKernelResults of the most recent run (for test.py)


def _get_nc():
    if "nc" not in _NC_CACHE:
        _NC_CACHE["nc"] = build_nc()
    return _NC_CACHE["nc"]


def kernel(**inputs) -> np.ndarray:
    global LAST_RESULT
    from concourse.bass_utils import run_bass_kernel_spmd

    nc = _get_nc()
    in_maps = _prep_core_inputs(inputs)
    res = run_bass_kernel_spmd(nc, in_maps, list(range(N_CORES)))
    LAST_RESULT = res
    out = np.stack([np.asarray(res.results[i]["out"]) for i in range(B)])
    return out.reshape(B, C, H, W).astype(np.float32)


# revision 16
# speedup vs baseline: 1.2447x; 1.0113x over previous
"""Trainium2 Bass kernel for nn_AttentionBlock (GroupNorm + single-head
self-attention + proj + residual), data-parallel over batch on 8 cores.

Contract: kernel(**inputs) takes the FULL unsharded inputs
  x (8, 256, 64, 64) f32, gn_scale (256,), gn_bias (256,),
  qkv_w (768, 256), qkv_b (768,), proj_w (256, 256), proj_b (256,)
and returns the FULL output (8, 256, 64, 64) f32.

Per-core plan (one sample per core):
  - x viewed as (C=256, N=4096) = (channels on partitions, tokens on free dim)
  - GroupNorm(8 groups) stats via bn_stats/bn_aggr + tiny indicator matmuls
  - xn cast to bf16; QKV as channel matmuls:
      Q, K produced in (d, n) layout  [d on partitions], stored fp8e4
      V produced in token-major (n, d) layout, stored fp8e4
  - attention runs over q-tiles of 512 columns x key-block PAIRS (2x128
    keys -> one DoubleRow contraction):
      scoresT[k, q] = sum_d K[d,k] Q[d,q]     (one DR matmul per kb)
      PT = exp(scoresT * scale)               (one ACT instr per kb-pair,
                                               [128, 1024] granularity)
      den[q]  += ones[k]^T PT                 (M=1 DR matmul, accumulated in
                                               a PSUM row across the q-tile;
                                               rows 0/32/64/96 by qt%4)
      o_un[d, q] += V[k,d]^T PT               (2 DR matmuls, f32 PSUM)
    -> NO per-step DVE/gpsimd work at all; the whole denominator rides on
    the PE (cost ~= 1 extra N=512 matmul per kb-pair).
  - finalize per q-tile (deferred into the next q-tile's first 2 steps):
      ob = bf16(o_un)                  (DVE copies; unblocks next tile's PV)
      rec = 1/den  -> DMA broadcast to 128 partitions via a DRAM bounce
      p = proj(ob); out = p*rec + proj_b + x   (normalize AFTER proj --
                                                rec commutes with the matmul)
PSUM budget (8 banks): scores [P,2,512]f32 x2bufs = 4, o_un x2 = 2,
den row bank = 1, proj psum = 1.
"""

import os
import sys

import numpy as np

for _p in (
    "/opt/trn_rl_repo",
    "/root/.axon_site",
    "/root/.axon_site/_ro/trn_rl_repo",
    "/root/.axon_site/_ro/pypackages",
):
    if os.path.isdir(_p) and _p not in sys.path:
        sys.path.append(_p)

import ml_dtypes  # noqa: E402

import concourse.bass as bass  # noqa: E402
import concourse.mybir as mybir  # noqa: E402
import concourse.tile as tile  # noqa: E402
from concourse import bacc  # noqa: E402

F32 = mybir.dt.float32
BF16 = mybir.dt.bfloat16
FP8 = mybir.dt.float8e4
AF = mybir.ActivationFunctionType
ALU = mybir.AluOpType
DR = mybir.MatmulPerfMode.DoubleRow

B, C, H, W = 8, 256, 64, 64
GROUPS = 8
EPS = 1e-5
P = 128
N_CORES = 8
ATT_SCALE = float(C) ** -0.5  # 1/16


def build_nc(n_tok=H * W):
    """Build the single-core Bass program (SPMD across 8 cores)."""
    CCH = C // P            # channel chunks (2)
    QT = 512                # q-tile width (one PSUM bank of f32)
    NQT = n_tok // QT       # number of q tiles (8)
    NKB = n_tok // P        # number of 128-token key blocks (32)
    NS = NKB // 2           # key-block PAIRS per q-tile (16)
    GSZ = C // GROUPS       # channels per group (32)

    nc = bacc.Bacc()

    # ---- DRAM I/O (per-core tensors; host shards batch over cores) ----
    x_d = nc.dram_tensor("x", [C, n_tok], F32, kind="ExternalInput")
    # qkv weights pre-scaled x32 on the host so they sit in fp8e4 normal
    # range; the 1/32 compensation rides on the PSUM->SBUF copies
    qkvw_d = nc.dram_tensor("qkv_wt", [CCH, P, 3 * C], FP8, kind="ExternalInput")
    qkbias_d = nc.dram_tensor("qk_bias", [4, P, 1], F32, kind="ExternalInput")
    projw_d = nc.dram_tensor("proj_wt", [CCH, P, C], BF16, kind="ExternalInput")
    projb_d = nc.dram_tensor("proj_b", [CCH, P, 1], F32, kind="ExternalInput")
    gnsc_d = nc.dram_tensor("gn_sc", [CCH, P, 1], F32, kind="ExternalInput")
    gnbi_d = nc.dram_tensor("gn_bi", [CCH, P, 1], F32, kind="ExternalInput")
    # group-sum indicator (zero-padded to M=128 so the matmul avoids the
    # 32-column tile-mode lowering): ind[t, c, g] = (t*128 + c) // 32 == g
    gnind_d = nc.dram_tensor("gn_ind", [CCH, P, P], F32, kind="ExternalInput")
    # channel-broadcast indicator, padded to K=128: ind2[t, g, c] nonzero only g<8
    gnind2_d = nc.dram_tensor("gn_ind2", [CCH, P, P], F32, kind="ExternalInput")
    out_d = nc.dram_tensor("out", [C, n_tok], F32, kind="ExternalOutput")

    with tile.TileContext(nc) as tc:
        with (
            tc.tile_pool(name="persist", bufs=1) as pp,
            tc.tile_pool(name="work", bufs=3) as wp,
            tc.tile_pool(name="ps_sc", bufs=2, space="PSUM") as psb,
            tc.tile_pool(name="ps_o", bufs=1, space="PSUM") as pso,
            tc.tile_pool(name="ps_den", bufs=1, space="PSUM") as psd,
        ):
            # ---------------- load weights / constants ----------------
            qkvw = pp.tile([P, CCH, 3 * C], FP8, tag="qkvw")
            nc.sync.dma_start(qkvw[:], qkvw_d.rearrange("t p o -> p t o"))
            projw = pp.tile([P, CCH, C], BF16, tag="projw")
            nc.sync.dma_start(projw[:], projw_d.rearrange("t p o -> p t o"))
            qkb = pp.tile([P, 4], F32, tag="qkb")
            nc.sync.dma_start(qkb[:], qkbias_d.rearrange("j p one -> p (j one)"))
            projb = pp.tile([P, CCH], F32, tag="projb")
            nc.sync.dma_start(projb[:], projb_d.rearrange("t p one -> p (t one)"))
            gnsc = pp.tile([P, CCH], F32, tag="gnsc")
            nc.sync.dma_start(gnsc[:], gnsc_d.rearrange("t p one -> p (t one)"))
            gnbi = pp.tile([P, CCH], F32, tag="gnbi")
            nc.sync.dma_start(gnbi[:], gnbi_d.rearrange("t p one -> p (t one)"))
            gnind = pp.tile([P, CCH, P], F32, tag="gnind")
            nc.sync.dma_start(gnind[:], gnind_d.rearrange("t p g -> p t g"))
            gnind2 = pp.tile([P, CCH, P], F32, tag="gnind2")
            nc.sync.dma_start(gnind2[:], gnind2_d.rearrange("t g c -> g t c"))
            # fp8 all-ones block: lhsT of the denominator matmuls (M=128 so
            # every PSUM partition gets the column sum -> broadcast for free;
            # M<128 would trigger the 32-column tile-mode lowering, which
            # crashes the exec unit)
            ones8 = pp.tile([P, 2, P], FP8, tag="ones8")
            nc.vector.memset(ones8[:], 1.0)

            # ---------------- load x, GroupNorm stats ----------------
            # per-1024-token chunks: bn_stats consumes each chunk as soon as
            # its DMA lands, so stats finish ~1 chunk after the last DMA
            x_sb = pp.tile([P, CCH, n_tok], F32, tag="x_sb")
            stats = pp.tile([P, CCH, 2], F32, tag="stats")
            XPC = max(1, n_tok // 1024)
            bn6s = []
            for t in range(CCH):
                bn6 = pp.tile([P, n_tok // 512, 6], F32, tag=f"bn6_{t}",
                              name=f"bn6_{t}")
                bn6s.append(bn6)
            for pc in range(XPC):
                xs = slice(pc * (n_tok // XPC), (pc + 1) * (n_tok // XPC))
                for t in range(CCH):
                    nc.sync.dma_start(x_sb[:, t, xs], x_d[t * P:(t + 1) * P, xs])
                    xv = x_sb[:, t, xs].rearrange("p (a b) -> p a b", b=512)
                    for a in range(2):
                        nc.vector.bn_stats(bn6s[t][:, 2 * pc + a], xv[:, a])
            for t in range(CCH):
                # mv = (mean, var) per partition
                nc.vector.bn_aggr(stats[:, t], bn6s[t][:])
                # stats col1 := mean^2 + var = E[x^2] (col0 stays mean)
                nc.vector.scalar_tensor_tensor(
                    out=stats[:, t, 1:2],
                    in0=stats[:, t, 0:1],
                    scalar=stats[:, t, 0:1],
                    in1=stats[:, t, 1:2],
                    op0=ALU.mult,
                    op1=ALU.add,
                )

            # group aggregation: gagg[g, j] = sum_{c in group g} stats[c, j]
            gagg_ps = psb.tile([P, 2, 512], F32, tag="sc", name="gagg_ps")
            for t in range(CCH):
                nc.tensor.matmul(
                    gagg_ps[:, 0, :2],
                    gnind[:, t],
                    stats[:, t],
                    start=(t == 0),
                    stop=(t == CCH - 1),
                )
            # per-group a = rstd, b = -mean * rstd   (divide sums by GSZ first)
            gab = pp.tile([P, 2], F32, tag="gab")
            nc.vector.memset(gab[:], 0.0)
            gmean = wp.tile([P, 1], F32, tag="gmean")
            gtmp = wp.tile([P, 1], F32, tag="gtmp")
            nc.vector.tensor_scalar_mul(gmean[:GROUPS], gagg_ps[:GROUPS, 0, 0:1], 1.0 / GSZ)
            nc.vector.tensor_scalar_mul(gtmp[:GROUPS], gagg_ps[:GROUPS, 0, 1:2], 1.0 / GSZ)
            # gtmp := mean^2 - E[x^2] = -var
            nc.vector.scalar_tensor_tensor(
                out=gtmp[:GROUPS],
                in0=gmean[:GROUPS],
                scalar=gmean[:GROUPS],
                in1=gtmp[:GROUPS],
                op0=ALU.mult,
                op1=ALU.subtract,
            )
            # std = sqrt(-1 * gtmp + eps)
            epsb = wp.tile([P, 1], F32, tag="epsb")
            nc.vector.memset(epsb[:], EPS)
            nc.scalar.activation(gtmp[:GROUPS], gtmp[:GROUPS], AF.Sqrt,
                                 bias=epsb[:GROUPS], scale=-1.0)
            nc.vector.reciprocal(gab[:GROUPS, 0:1], gtmp[:GROUPS])  # a = rstd
            # b = -(mean * rstd)
            nc.vector.tensor_mul(gtmp[:GROUPS], gmean[:GROUPS], gab[:GROUPS, 0:1])
            nc.vector.tensor_scalar_mul(gab[:GROUPS, 1:2], gtmp[:GROUPS], -1.0)

            # broadcast (a, b) back to channels: chab[c, j] = gab[g(c), j]
            xn = pp.tile([P, CCH, n_tok], FP8, tag="xn")
            for t in range(CCH):
                chab_ps = psb.tile([P, 2, 512], F32, tag="sc", name=f"chab_ps{t}")[:, 0]
                nc.tensor.matmul(chab_ps[:, :2], gnind2[:, t], gab[:],
                                 start=True, stop=True)
                # mult_c = a * gamma_c ; add_c = b * gamma_c + beta_c
                chm = pp.tile([P, 1], F32, tag=f"chm{t}", name=f"chm{t}")
                cha = pp.tile([P, 1], F32, tag=f"cha{t}", name=f"cha{t}")
                nc.vector.tensor_mul(chm[:], chab_ps[:, 0:1], gnsc[:, t, None])
                nc.vector.scalar_tensor_tensor(
                    out=cha[:],
                    in0=chab_ps[:, 1:2],
                    scalar=gnsc[:, t, None],
                    in1=gnbi[:, t, None],
                    op0=ALU.mult,
                    op1=ALU.add,
                )
                # xn = x * mult + add   (bf16 out); t=0 on ACT, t=1 on DVE
                if t == 0:
                    nc.scalar.activation(xn[:, t], x_sb[:, t], AF.Identity,
                                         bias=cha[:], scale=chm[:])
                else:
                    nc.vector.tensor_scalar(xn[:, t], x_sb[:, t], chm[:], cha[:],
                                            op0=ALU.mult, op1=ALU.add)

            # ---------------- QKV ----------------
            # Q, K in (d, n) layout; j = 0,1 -> Q chunks; 2,3 -> K chunks.
            # fp8 DoubleRow matmuls contract both channel chunks at once;
            # the PSUM->SBUF copies (which also apply the 1/32 weight-scale
            # compensation + bias) alternate ACT / DVE to halve the wall.
            # v_bias is folded into proj_b on the host (proj_w @ v_bias).
            WS = 1.0 / 32.0
            qk = pp.tile([P, 4, n_tok], FP8, tag="qk")
            for j in range(4):
                for np2 in range(NQT // 2):
                    ns = slice(np2 * 2 * QT, (np2 + 1) * 2 * QT)
                    ps = psb.tile([P, 2, 512], F32, tag="sc",
                                  name=f"qk{j}_{np2}")
                    for half in range(2):
                        nsh = slice(np2 * 2 * QT + half * QT,
                                    np2 * 2 * QT + (half + 1) * QT)
                        nc.tensor.matmul(
                            ps[:, half],
                            qkvw[:, 0:2, j * P:(j + 1) * P],
                            xn[:, 0:2, nsh],
                            start=True, stop=True, perf_mode=DR,
                        )
                    if (j * (NQT // 2) + np2) % 2 == 0:
                        nc.scalar.activation(
                            qk[:, j, ns], ps.rearrange("p a b -> p (a b)"),
                            AF.Identity,
                            bias=qkb[:, j, None],
                            scale=WS,
                        )
                    else:
                        nc.vector.tensor_scalar(
                            qk[:, j, ns], ps.rearrange("p a b -> p (a b)"),
                            WS, qkb[:, j, None],
                            op0=ALU.mult, op1=ALU.add,
                        )
            # V token-major: v_sb[:, kb, d] holds V[token kb*128+p, d]
            v_sb = pp.tile([P, NKB, C], FP8, tag="v_sb")
            for kbp in range(NKB // 2):
                ps = psb.tile([P, 2, 512], F32, tag="sc", name=f"v{kbp}")
                for k2 in range(2):
                    kb = 2 * kbp + k2
                    nc.tensor.matmul(
                        ps[:, k2, :C],
                        xn[:, 0:2, kb * P:(kb + 1) * P],
                        qkvw[:, 0:2, 2 * C:3 * C],
                        start=True, stop=True, perf_mode=DR,
                    )
                if kbp % 2 == 0:
                    nc.scalar.activation(
                        v_sb[:, 2 * kbp:2 * kbp + 2],
                        ps[:, :, :C], AF.Copy, scale=WS,
                    )
                else:
                    nc.vector.tensor_scalar_mul(
                        v_sb[:, 2 * kbp:2 * kbp + 2], ps[:, :, :C], WS,
                    )

            # ---------------- attention + proj + residual ----------------
            # q-tiles of 512 columns; per step = one key-block PAIR.
            # den accumulates on the PE into one PSUM row (qt%4 -> 0/32/64/96).
            den_all = psd.tile([P, QT], F32, tag="den", name="den_all")

            def emit_den_pv(pt, s, o_ps, qt):
                nc.tensor.matmul(den_all[:], ones8[:], pt[:],
                                 start=(s == 0), stop=(s == NS - 1),
                                 perf_mode=DR)
                for ch in range(2):
                    nc.tensor.matmul(
                        o_ps[ch][:],
                        v_sb[:, 2 * s:2 * s + 2, ch * P:(ch + 1) * P],
                        pt[:],
                        start=(s == 0), stop=(s == NS - 1), perf_mode=DR)

            def fin_stage1(qt, o_ps):
                # recip first (releases the den bank for the next q-tile's
                # den matmul); then the o_un bf16 copies (DVE: gpsimd has no
                # PSUM port) release the o_ps banks for the next tile's PV.
                rec_bc = wp.tile([P, QT], F32, tag="rec_bc", bufs=2,
                                 name=f"rec_bc_{qt}")
                nc.vector.reciprocal_approx_fast(rec_bc[:], den_all[:])
                obs = []
                for ch in range(2):
                    ob = wp.tile([P, QT], BF16, tag=f"ob{ch}", bufs=2,
                                 name=f"ob{ch}_{qt}")
                    nc.vector.tensor_copy(ob[:], o_ps[ch][:])
                    obs.append(ob)
                return obs, rec_bc

            def fin_stage2(qt, obs, rec_bc):
                qs = slice(qt * QT, (qt + 1) * QT)
                for t in range(CCH):
                    p_ps = psb.tile([P, QT], F32, tag="pp", bufs=1,
                                    name=f"p_{qt}_{t}")
                    nc.tensor.matmul(p_ps[:], projw[:, 0, t * P:(t + 1) * P],
                                     obs[0][:], start=True, stop=False)
                    nc.tensor.matmul(p_ps[:], projw[:, 1, t * P:(t + 1) * P],
                                     obs[1][:], start=False, stop=True)
                    # out = p*rec + proj_b + x  (normalize after proj)
                    tmp = wp.tile([P, QT], F32, tag="tmp", bufs=2)
                    nc.vector.tensor_mul(tmp[:], p_ps[:], rec_bc[:])
                    res = wp.tile([P, QT], F32, tag="res", bufs=3)
                    nc.vector.scalar_tensor_tensor(
                        out=res[:],
                        in0=tmp[:],
                        scalar=projb[:, t, None],
                        in1=x_sb[:, t, qs],
                        op0=ALU.add,
                        op1=ALU.add,
                    )
                    nc.sync.dma_start(out_d[t * P:(t + 1) * P, qs], res[:])

            prev = None       # (pt, s, o_ps, qt) awaiting den+PV emission
            pending = None    # (qt, o_ps) awaiting finalize
            hold = None       # (obs, rec_bc) between fin stages
            for qt in range(NQT):
                qs = slice(qt * QT, (qt + 1) * QT)
                o_ps = [pso.tile([P, QT], F32, tag=f"o{ch}",
                                 name=f"o{ch}_{qt}") for ch in range(2)]
                for s in range(NS):
                    pt = wp.tile([P, 2, QT], FP8, tag="pt", bufs=4)
                    s_ps = psb.tile([P, 2, QT], F32, tag="sc",
                                    name=f"s_{qt}_{s}")
                    for k2 in range(2):
                        kb = 2 * s + k2
                        nc.tensor.matmul(
                            s_ps[:, k2],
                            qk[:, 2:4, kb * P:(kb + 1) * P],
                            qk[:, 0:2, qs],
                            start=True, stop=True, perf_mode=DR)
                    nc.scalar.activation(
                        pt.rearrange("p a b -> p (a b)"),
                        s_ps.rearrange("p a b -> p (a b)"),
                        AF.Exp, scale=ATT_SCALE)
                    if prev is not None:
                        emit_den_pv(*prev)
                    prev = (pt, s, o_ps, qt)
                    if s == 0 and pending is not None:
                        hold = fin_stage1(*pending)
                    elif s == 1 and pending is not None:
                        fin_stage2(pending[0], *hold)
                        pending = None
                        hold = None
                pending = (qt, o_ps)
            emit_den_pv(*prev)
            hold = fin_stage1(*pending)
            fin_stage2(pending[0], *hold)

    nc.finalize()
    return nc


# ---------------------------------------------------------------------------
# host side
# ---------------------------------------------------------------------------

def _prep_core_inputs(inputs, n_tok=H * W):
    """Build the per-core in_maps (shared weight tensors + per-core x)."""
    CCH = C // P
    f32 = np.float32
    bf16 = ml_dtypes.bfloat16
    fp8 = ml_dtypes.float8_e4m3

    x = np.asarray(inputs["x"], f32).reshape(B, C, n_tok)
    gn_scale = np.asarray(inputs["gn_scale"], f32)
    gn_bias = np.asarray(inputs["gn_bias"], f32)
    qkv_w = np.asarray(inputs["qkv_w"], f32)
    qkv_b = np.asarray(inputs["qkv_b"], f32)
    proj_w = np.asarray(inputs["proj_w"], f32)
    proj_b = np.asarray(inputs["proj_b"], f32)

    # x32 lifts the ~0.02-scale weights into fp8e4 normal range; the kernel
    # multiplies the QKV PSUM results by 1/32
    qkv_wt = (np.ascontiguousarray(qkv_w.T) * 32.0).reshape(
        CCH, P, 3 * C).astype(fp8)
    qk_bias = qkv_b[:2 * C].reshape(4, P, 1).astype(f32).copy()
    v_bias = qkv_b[2 * C:].astype(f32)
    proj_wt = np.ascontiguousarray(proj_w.T).reshape(CCH, P, C).astype(bf16)
    # v_bias folds through the attention average (sum_k pt*vb / den = vb)
    # and the linear proj into the proj bias
    proj_bt = (proj_b + proj_w @ v_bias).reshape(CCH, P, 1).astype(f32)
    gn_sc = gn_scale.reshape(CCH, P, 1).astype(f32)
    gn_bi = gn_bias.reshape(CCH, P, 1).astype(f32)

    ch = np.arange(C)
    gn_ind = np.zeros((CCH, P, P), f32)
    gn_ind[ch // P, ch % P, ch // (C // GROUPS)] = 1.0
    gn_ind2 = np.zeros((CCH, P, P), f32)
    for t in range(CCH):
        gn_ind2[t, :GROUPS, :] = gn_ind[t, :, :GROUPS].T

    shared = {
        "qkv_wt": qkv_wt,
        "qk_bias": qk_bias,
        "proj_wt": proj_wt,
        "proj_b": proj_bt,
        "gn_sc": gn_sc,
        "gn_bi": gn_bi,
        "gn_ind": gn_ind,
        "gn_ind2": gn_ind2,
    }
    return [dict(shared, x=np.ascontiguousarray(x[i])) for i in range(B)]


_NC_CACHE = {}
LAST_RESULT = None  # BassKernelResults of the most recent run (for test.py)


def _get_nc():
    if "nc" not in _NC_CACHE:
        _NC_CACHE["nc"] = build_nc()
    return _NC_CACHE["nc"]


def kernel(**inputs) -> np.ndarray:
    global LAST_RESULT
    from concourse.bass_utils import run_bass_kernel_spmd

    nc = _get_nc()
    in_maps = _prep_core_inputs(inputs)
    res = run_bass_kernel_spmd(nc, in_maps, list(range(N_CORES)))
    LAST_RESULT = res
    out = np.stack([np.asarray(res.results[i]["out"]) for i in range(B)])
    return out.reshape(B, C, H, W).astype(np.float32)
